# revision 12
# baseline (speedup 1.0000x reference)
"""Trainium2 Bass kernel for nn_Cross_MultiAttention (8-head cross attention).

Sharding: one attention head per NeuronCore (8 heads / 8 cores).

Host folds the shared 1x1 input conv into each head's q/k/v projections
(Aq = wq_h @ w_in etc.), so each core:
  - projects q/k/v for its head directly from (x+pos) / (context+pos),
  - computes the full 5000x5000 attention for its head with scores kept
    TRANSPOSED (keys on partitions, queries on the free dim). Softmax is
    max-free (|scores/16| < ~4) and the denominator comes from an appended
    ones-column in V, so no cross-partition reductions are needed.
  - The probability tiles for a whole 1024-query stripe are kept in SBUF,
    and the P@V pass for stripe w-1 is interleaved with the QK^T pass for
    stripe w, so the tensor engine never waits on softmax.
  - applies its head's slice of the output projection to the UNNORMALIZED
    attention output and exports the per-query softmax denominator row.
Host divides each partial [256, 5000] by its denominator, sums the 8
partials, adds b_out, reshapes to [256, 50, 100].

dtypes: fp32 in DRAM except the mask (fp16 0/1); on-chip the big matmuls
run in float32r (full-rate PE mode), probabilities/V in fp16 (bounded).
"""

import numpy as np

import concourse.bacc as bacc
import concourse.tile as tile
import concourse.mybir as mybir
from concourse.bass_utils import run_bass_kernel_spmd

F32 = mybir.dt.float32
F32R = mybir.dt.float32r  # fp32 bits, full-rate PE streaming mode (rounded)
F16 = mybir.dt.float16
F8 = mybir.dt.float8e4
AF = mybir.ActivationFunctionType

EMB = 256
HEADS = 8
DEPTH = 32
IN_CH = 256
H, W = 50, 100
N_TOK = H * W  # 5000
SCALE = EMB ** (-0.5)  # 1/16


def _tiles(total, size):
    out = []
    p = 0
    while p < total:
        out.append((p, min(size, total - p)))
        p += size
    return out


def build_nc(n_tok=N_TOK, num_devices=8, wsz=1024, jsz=128):
    """Build the Bass module (same SPMD program for every core)."""
    nc = bacc.Bacc("TRN2", target_bir_lowering=False, debug=False,
                   num_devices=num_devices)

    D = DEPTH
    xp_d = nc.dram_tensor("xp", (IN_CH, n_tok), F32R, kind="ExternalInput").ap()
    cp_d = nc.dram_tensor("cp", (IN_CH, n_tok), F32R, kind="ExternalInput").ap()
    nmT_d = nc.dram_tensor("nmT", (n_tok, n_tok), F16, kind="ExternalInput").ap()
    AqT_d = nc.dram_tensor("AqT", (IN_CH, 4 * D), F32R, kind="ExternalInput").ap()
    cq_d = nc.dram_tensor("cq", (4 * D, 1), F32, kind="ExternalInput").ap()
    AkT_d = nc.dram_tensor("AkT", (IN_CH, 4 * D), F32R, kind="ExternalInput").ap()
    ck_d = nc.dram_tensor("ck", (4 * D, 1), F32, kind="ExternalInput").ap()
    AvT_d = nc.dram_tensor("AvT", (IN_CH, D), F32, kind="ExternalInput").ap()
    cvb_d = nc.dram_tensor("cvb", (128, D), F32, kind="ExternalInput").ap()
    woT_d = nc.dram_tensor("woT", (D, EMB), F32R, kind="ExternalInput").ap()
    y_d = nc.dram_tensor("y", (EMB, n_tok), F32, kind="ExternalOutput").ap()
    dn_d = nc.dram_tensor("dn", (1, n_tok), F32, kind="ExternalOutput").ap()

    ntiles = _tiles(n_tok, 512)   # 512-wide tiles (projections)
    wtiles = _tiles(n_tok, wsz)   # wide query stripes for the attention loop
    jtiles = _tiles(n_tok, jsz)   # key tiles (partition dim of scores)
    NJ = len(jtiles)
    NW = len(wtiles)

    with tile.TileContext(nc) as tc:
        with (
            tc.tile_pool(name="persist", bufs=1) as persist,
            tc.tile_pool(name="consts", bufs=1) as consts,
        ):
            # ---- constants to SBUF ----
            AqT_sb = consts.tile([128, 2, 4 * D], F32R)
            AkT_sb = consts.tile([128, 2, 4 * D], F32R)
            AvT_sb = consts.tile([128, 2, D], F32)
            for ct in range(2):
                nc.sync.dma_start(AqT_sb[:, ct, :], AqT_d[ct * 128:(ct + 1) * 128, :])
                nc.sync.dma_start(AkT_sb[:, ct, :], AkT_d[ct * 128:(ct + 1) * 128, :])
                nc.sync.dma_start(AvT_sb[:, ct, :], AvT_d[ct * 128:(ct + 1) * 128, :])
            cq_sb = consts.tile([4 * D, 1], F32)
            nc.sync.dma_start(cq_sb[:, :], cq_d[:, :])
            ck_sb = consts.tile([4 * D, 1], F32)
            nc.sync.dma_start(ck_sb[:, :], ck_d[:, :])
            cvb_sb = consts.tile([128, D], F32)
            nc.sync.dma_start(cvb_sb[:, :], cvb_d[:, :])
            woT_sb = consts.tile([D, EMB], F32R)
            nc.sync.dma_start(woT_sb[:, :], woT_d[:, :])

            # ---- persistent activations ----
            qT = persist.tile([4 * D, n_tok], F16)
            kT = persist.tile([4 * D, n_tok], F16)
            v_sb = persist.tile([128, NJ, D + 1], F16)  # [j % 128, jt, d | ones]
            ones_stage = consts.tile([128, NJ], F32)
            nc.any.memset(ones_stage[:, :], 1.0)
            nc.vector.tensor_copy(v_sb[:, :, D], ones_stage[:, :])
            # probability stripe: all NJ key-tiles for one query stripe
            p_store = persist.tile([128, NJ, wsz], F16)

            # ---- stage 1: project q/k/v straight from (x|context)+pos ----
            with (
                tc.tile_pool(name="proj_in", bufs=3) as proj_in,
                tc.tile_pool(name="qk_ps", bufs=2, space="PSUM") as qk_ps,
                tc.tile_pool(name="v_ps", bufs=2, space="PSUM") as v_ps,
            ):
                for (n0, ns) in ntiles:
                    img_t = proj_in.tile([128, 2, 512], F32R, name="img_t")
                    for ct in range(2):
                        nc.sync.dma_start(
                            img_t[:, ct, :ns],
                            xp_d[ct * 128:(ct + 1) * 128, n0:n0 + ns])
                    qps = qk_ps.tile([4 * D, 512], F32, name="qps")
                    for ct in range(2):
                        nc.tensor.matmul(qps[:, :ns], AqT_sb[:, ct, :],
                                         img_t[:, ct, :ns],
                                         start=(ct == 0), stop=(ct == 1))
                    nc.vector.tensor_scalar_add(qT[:, n0:n0 + ns], qps[:, :ns],
                                                cq_sb[:, :])

                for (n0, ns) in ntiles:
                    img_t = proj_in.tile([128, 2, 512], F32R, name="img_t")
                    for ct in range(2):
                        nc.sync.dma_start(
                            img_t[:, ct, :ns],
                            cp_d[ct * 128:(ct + 1) * 128, n0:n0 + ns])
                    kps = qk_ps.tile([4 * D, 512], F32, name="qps")
                    for ct in range(2):
                        nc.tensor.matmul(kps[:, :ns], AkT_sb[:, ct, :],
                                         img_t[:, ct, :ns],
                                         start=(ct == 0), stop=(ct == 1))
                    nc.vector.tensor_scalar_add(kT[:, n0:n0 + ns], kps[:, :ns],
                                                ck_sb[:, :])
                    # v projection for the j-tiles inside this 512 stripe
                    for (jj0, jjs) in _tiles(ns, jsz):
                        jt = (n0 + jj0) // jsz
                        vps = v_ps.tile([128, D], F32, name="vps")
                        for ct in range(2):
                            nc.tensor.matmul(
                                vps[:jjs, :],
                                img_t[:, ct, jj0:jj0 + jjs].bitcast(F32),
                                AvT_sb[:, ct, :],
                                start=(ct == 0), stop=(ct == 1))
                        nc.vector.tensor_add(v_sb[:jjs, jt, 0:D], vps[:jjs, :],
                                             cvb_sb[:jjs, :])

            # ---- stage 2: pipelined attention + output projection ----
            with (
                tc.tile_pool(name="s_ps", bufs=3, space="PSUM") as s_ps_pool,
                tc.tile_pool(name="av_ps", bufs=1, space="PSUM") as av_ps_pool,
                tc.tile_pool(name="m_sb", bufs=3) as m_pool,
                tc.tile_pool(name="out_sb", bufs=2) as out_pool,
            ):
                def epilogue(av, i0p, iszp):
                    # unnormalized head output, denominator row, partial
                    # output projection for a finished stripe
                    unn = out_pool.tile([D + 1, wsz], F32R, name="unn")
                    nc.vector.tensor_copy(unn[:, :iszp], av[:, :iszp])
                    nc.sync.dma_start(dn_d[:, i0p:i0p + iszp],
                                      unn[D:D + 1, :iszp].bitcast(F32))
                    for c2 in range(2):
                        for (h0, hs) in _tiles(iszp, 512):
                            yps = s_ps_pool.tile([128, 512], F32, name="yps",
                                                 tag="s")
                            nc.tensor.matmul(
                                yps[:, :hs],
                                woT_sb[:, c2 * 128:(c2 + 1) * 128],
                                unn[0:D, h0:h0 + hs],
                                start=True, stop=True)
                            ysb = out_pool.tile([128, 512], F32, name="ysb")
                            nc.vector.tensor_copy(ysb[:, :hs], yps[:, :hs])
                            nc.sync.dma_start(
                                y_d[c2 * 128:(c2 + 1) * 128,
                                    i0p + h0:i0p + h0 + hs],
                                ysb[:, :hs])

                av = None
                pending = None  # (av, i0, isz) of the just-finished stripe
                for w in range(NW + 1):
                    if w >= 1:
                        i0p, iszp = wtiles[w - 1]
                        av = av_ps_pool.tile([D + 1, wsz], F32, name="av")
                    for jt, (j0, js) in enumerate(jtiles):
                        if w >= 1:
                            # P@V' for the PREVIOUS stripe (operands ready)
                            for hi, (h0, hs) in enumerate(_tiles(iszp, 512)):
                                mm = nc.tensor.matmul(
                                    av[:, h0:h0 + hs],
                                    v_sb[:js, jt, :],
                                    p_store[:js, jt, h0:h0 + hs],
                                    start=(jt == 0), stop=(jt == NJ - 1))
                                if hi > 0:
                                    mm.ldweights = False
                        if w < NW:
                            i0, isz = wtiles[w]
                            s = s_ps_pool.tile([128, wsz], F32, name="s")
                            for hi, (h0, hs) in enumerate(_tiles(isz, 512)):
                                mm = nc.tensor.matmul(
                                    s[:js, h0:h0 + hs],
                                    kT[:, j0:j0 + js],
                                    qT[:, i0 + h0:i0 + h0 + hs],
                                    start=True, stop=True)
                                if hi > 0:
                                    mm.ldweights = False
                            nc.scalar.activation(
                                p_store[:js, jt, :isz], s[:js, :isz],
                                AF.Exp, scale=float(SCALE) / 4.0)
                            m = m_pool.tile([128, wsz], F16, name="m")
                            nc.sync.dma_start(m[:js, :isz],
                                              nmT_d[j0:j0 + js, i0:i0 + isz])
                            nc.vector.tensor_mul(p_store[:js, jt, :isz],
                                                 p_store[:js, jt, :isz],
                                                 m[:js, :isz])
                        if jt == 4 and pending is not None:
                            epilogue(*pending)
                            pending = None
                    if w >= 1:
                        pending = (av, i0p, iszp)
                if pending is not None:
                    epilogue(*pending)

    nc.compile()
    return nc


def make_pos(row_embed, col_embed):
    """[EMB, H*W]; first half col embeds, second half row embeds."""
    d2 = row_embed.shape[1]
    pos = np.empty((EMB, H, W), np.float32)
    pos[:d2] = col_embed[:W].T[:, None, :]      # [d2, 1, W] -> broadcast H
    pos[d2:] = row_embed[:H].T[:, :, None]      # [d2, H, 1] -> broadcast W
    return pos.reshape(EMB, H * W)


def make_in_maps(x, context, pad_mask, row_embed, col_embed, w_in, b_in,
                 wq, bq, wk, bk, wv, bv, w_out, n_heads=HEADS):
    f8 = np.float64
    x = np.asarray(x, np.float32)
    context = np.asarray(context, np.float32)
    pad_mask = np.asarray(pad_mask)
    row_embed = np.asarray(row_embed, np.float32)
    col_embed = np.asarray(col_embed, np.float32)
    w_in = np.asarray(w_in, f8)
    b_in = np.asarray(b_in, f8)
    w_out = np.asarray(w_out, np.float32)
    wq, bq = np.asarray(wq, f8), np.asarray(bq, f8)
    wk, bk = np.asarray(wk, f8), np.asarray(bk, f8)
    wv, bv = np.asarray(wv, f8), np.asarray(bv, f8)

    pos = make_pos(row_embed, col_embed)
    xp = np.ascontiguousarray(x.reshape(EMB, N_TOK) + pos)
    cp = np.ascontiguousarray(context.reshape(EMB, N_TOK) + pos)
    nmT = np.ascontiguousarray((~pad_mask[0]).T).astype(np.float16)

    shared = {"xp": xp, "cp": cp, "nmT": nmT}
    in_maps = []
    for h in range(n_heads):
        sl = slice(h * DEPTH, (h + 1) * DEPTH)
        Aq = wq[sl] @ w_in          # [D, IN_CH]
        cq = wq[sl] @ b_in + bq[sl]
        Ak = wk[sl] @ w_in
        ck = wk[sl] @ b_in + bk[sl]
        Av = wv[sl] @ w_in
        cv = wv[sl] @ b_in + bv[sl]
        f32c = lambda a: np.ascontiguousarray(a.astype(np.float32))
        in_maps.append(dict(
            shared,
            AqT=f32c(np.tile(Aq.T, (1, 4))),
            cq=f32c(np.tile(cq.reshape(DEPTH, 1), (4, 1))),
            AkT=f32c(np.tile(Ak.T, (1, 4))),
            ck=f32c(np.tile(ck.reshape(DEPTH, 1), (4, 1))),
            AvT=f32c(Av.T),
            cvb=f32c(np.broadcast_to(cv, (128, DEPTH))),
            woT=np.ascontiguousarray(w_out[:, sl].T),
        ))
    return in_maps


_CACHE = {}


def kernel(x, context, pad_mask, row_embed, col_embed, w_in, b_in,
           wq, bq, wk, bk, wv, bv, w_out, b_out):
    if "nc" not in _CACHE:
        _CACHE["nc"] = build_nc()
    nc = _CACHE["nc"]
    in_maps = make_in_maps(x, context, pad_mask, row_embed, col_embed,
                           w_in, b_in, wq, bq, wk, bk, wv, bv, w_out)
    res = run_bass_kernel_spmd(nc, in_maps, core_ids=list(range(HEADS)))
    y = np.zeros((EMB, N_TOK), np.float64)
    for c in range(HEADS):
        r = res.results[c]
        y += r["y"].astype(np.float64) / r["dn"].astype(np.float64)
    y = (y + np.asarray(b_out, np.float64)[:, None]).astype(np.float32)
    return y.reshape(EMB, H, W)
